# revision 44
# baseline (speedup 1.0000x reference)
"""Trainium2 Bass kernel for nn_AdditiveDTMGP.

3-stage additive GP network. Per stage:
    k = exp(-|h[b,f] - U[m]|)           ([B,F,63])
    feat = k @ Rinv
    out[b,o] = einsum('bfn,fon->bo', feat, Wm) + bias,
    Wm = mu + softplus(rho)*eps
plus a scalar KL over the variational params.

Key algebraic rewrite: fold Rinv into the weights
    out[b,o] = sum_{f,m} k[b,f,m] * W'[f,o,m]
    W'[f,o,m] = sum_n Rinv[m,n] * Wm[f,o,n]
which eliminates the [B,F,63]x[63,63] feature einsum entirely.

On-chip layout per stage (per core, B_local = 2048):
  - chunks of 128 partitions: feature 2c on p in [0,63), feature 2c+1 on
    p in [64,127), rows 63/127 dead (W' rows zeroed)
  - PE broadcast-matmul (host-built selector E) replicates h^T rows -> PSUM
  - DVE fused tensor_scalar: (t - U[p]) then abs_max(.,0) = |t - U|
  - ACT Exp(scale=-1): k = exp(-|t-U|)
  - PE contraction matmul (k x W'_chunk) accumulated over chunks in PSUM
  - ACT Identity(bias) evacuates PSUM -> h_next (bias folded in)

Sharding: pure data parallelism over batch (16384 -> 8 x 2048), params
replicated; KL computed redundantly per core (core 0's value returned).
"""

from contextlib import ExitStack

import numpy as np

import concourse.bacc as bacc
import concourse.bass as bass
import concourse.mybir as mybir
import concourse.tile as tile
from concourse import bass_utils

# ---- problem constants (hardcoded; kernel.py must be self-contained) ----
B_FULL = 16384
N_CORES = 8
BL = B_FULL // N_CORES  # 2048 batch per core
M = 63                  # dyadic design points
BB = 1024               # batch block size in the main loop
STAGES = [(32, 16), (16, 16), (16, 1)]  # (F, O)
FP = mybir.dt.float32
FR_B = mybir.dt.float32   # fp32: fp32r diverges on HW when interleaved
FR_K = mybir.dt.float32   # fp32: fp32r diverges on HW when interleaved

AF = mybir.ActivationFunctionType
ALU = mybir.AluOpType

N_KL = sum(F * O * M + F * O for F, O in STAGES)


def host_constants():
    """Constant input tensors computed on the host (replicated per core)."""
    ident = np.eye(128, dtype=np.float32)
    ones_col = np.ones((128, 1), dtype=np.float32)
    zeros_t = np.zeros((128, 16), dtype=np.float32)
    kl_off = np.full((1, 1), -0.5 * N_KL, dtype=np.float32)
    return {"ident": ident, "ones_col": ones_col,
            "zeros_t": zeros_t, "kl_off": kl_off}


def _e_all(U):
    """Paired broadcast selectors: cols [0:2048] produce d = h_f - U[m],
    cols [2048:4096] produce -d. Row 32 multiplies the constant ones row
    of h; rows 16..31 are zero for the F=16 stages (h junk killers)."""
    u_col = np.zeros(128, dtype=np.float32)
    u_col[0:M] = U
    u_col[64:64 + M] = U
    e = np.zeros((33, 16 * 128), dtype=np.float32)
    for c in range(16):
        e[2 * c, c * 128:c * 128 + M] = 1.0
        e[2 * c + 1, c * 128 + 64:c * 128 + 64 + M] = 1.0
        e[32, c * 128:(c + 1) * 128] = -u_col
    return e


def build_nc(debug: bool = False) -> bass.Bass:
    nc = bacc.Bacc("TRN2", debug=debug, target_bir_lowering=False,
                   num_devices=N_CORES)

    x = nc.dram_tensor("x", [BL, 32], FP, kind="ExternalInput")
    Rinv = nc.dram_tensor("Rinv", [M, M], FP, kind="ExternalInput")
    consts = {
        "E_all": nc.dram_tensor("E_all", [33, 16 * 128], FR_B,
                                kind="ExternalInput"),
        "ident": nc.dram_tensor("ident", [128, 128], FP,
                                kind="ExternalInput"),
        "ones_col": nc.dram_tensor("ones_col", [128, 1], FP,
                                   kind="ExternalInput"),
        "zeros_t": nc.dram_tensor("zeros_t", [128, 16], FP,
                                  kind="ExternalInput"),
        "kl_off": nc.dram_tensor("kl_off", [1, 1], FP,
                                 kind="ExternalInput"),
    }
    params = {}
    for si, (F, O) in enumerate(STAGES, start=1):
        for nm in ("mu_w", "rho_w", "eps_w"):
            params[f"{nm}{si}"] = nc.dram_tensor(
                f"{nm}{si}", [F, O, M], FP, kind="ExternalInput")
        for nm in ("mu_b", "rho_b", "eps_b"):
            params[f"{nm}{si}"] = nc.dram_tensor(
                f"{nm}{si}", [F, O], FP, kind="ExternalInput")
    out = nc.dram_tensor("out", [1, BL], FP, kind="ExternalOutput")
    kl = nc.dram_tensor("kl", [1, 1], FP, kind="ExternalOutput")

    with tile.TileContext(nc) as tc:
        _build(tc, x, Rinv, consts, params, out, kl)
    nc.compile()
    return nc


def _build(tc, x, Rinv, consts, params, out, kl):
    nc = tc.nc
    with ExitStack() as ctx:
        const = ctx.enter_context(tc.tile_pool(name="const", bufs=1))
        hbuf = ctx.enter_context(tc.tile_pool(name="hbuf", bufs=1))
        wp_pool = ctx.enter_context(tc.tile_pool(name="wp", bufs=1))
        pp = ctx.enter_context(tc.tile_pool(name="prep", bufs=1))

        # ---------- host constants -> SBUF ----------
        E_sb = const.tile([33, 16 * 128], FR_B, tag="E_sb")
        nc.sync.dma_start(out=E_sb[:], in_=consts["E_all"][:, :])
        ident = const.tile([128, 128], FP, tag="ident")
        nc.sync.dma_start(out=ident[:], in_=consts["ident"][:, :])
        ones_col = const.tile([128, 1], FP, tag="ones_col")
        nc.sync.dma_start(out=ones_col[:], in_=consts["ones_col"][:, :])
        zeros_t = const.tile([128, 16], FP, tag="zeros_t")
        nc.sync.dma_start(out=zeros_t[:], in_=consts["zeros_t"][:, :])
        kl_off = const.tile([1, 1], FP, tag="kl_off")
        nc.sync.dma_start(out=kl_off[:], in_=consts["kl_off"][:, :])

        # klv accumulates per-partition KL partial sums
        klv = const.tile([128, 1], FP, tag="klv")
        nc.vector.tensor_copy(klv[:], zeros_t[:, 0:1])

        # persistent stage activations (h0 = x^T), fp32r for fast matmul.
        # Row 32 is a constant ones row (multiplied by E's -+U row); rows
        # F..31 are zeroed so the E selectors see no garbage.
        h_tiles = [hbuf.tile([33, BL], FR_B, tag=f"h{si}", name=f"h{si}")
                   for si in range(len(STAGES))]
        h3 = hbuf.tile([1, BL], FP, tag="h3")
        for ht in h_tiles[1:]:
            nc.vector.memset(ht[:].bitcast(FP), 0.0)
        for ht in h_tiles:
            nc.vector.memset(ht[32:33, :].bitcast(FP), 1.0)

        wp_tiles = []    # per stage: list of [128, O] chunk weight tiles
        bias_tiles = []  # per stage: [O, 1]

        with ExitStack() as prep_ctx:
            pps = prep_ctx.enter_context(
                tc.tile_pool(name="prep_ps", bufs=3, space="PSUM"))

            # --- x^T via PE transposes (one DMA, [p, (i f)] layout) ---
            xT = h_tiles[0]
            x_big = pp.tile([128, 16 * 32], FP, tag="x_big")
            # x_big[p, i*32+f] = x[i*128+p, f]
            nc.sync.dma_start(
                out=x_big[:],
                in_=bass.AP(x, 0, [[32, 128], [128 * 32, 16], [1, 32]]))
            for g in range(4):  # 4 psum tiles, 4 transposes each
                xps = pps.tile([32, 512], FP, tag="ps", name=f"xps{g}")
                for t in range(4):
                    i = g * 4 + t
                    nc.tensor.transpose(xps[:, t * 128:(t + 1) * 128],
                                        x_big[:, i * 32:(i + 1) * 32],
                                        ident[:])
                nc.scalar.copy(xT[0:32, g * 512:(g + 1) * 512], xps[:])

            # --- Rinv^T ---
            rv = pp.tile([M, M], FP, tag="rv")
            nc.sync.dma_start(out=rv[:], in_=Rinv[:, :])
            rps = pps.tile([M, M], FP, tag="ps", name="rps")
            nc.tensor.transpose(rps[:], rv[:], ident[0:M, 0:M])
            rvT = const.tile([M, M], FP, tag="rvT")
            nc.scalar.copy(rvT[:], rps[:])

            # --- per stage: W' chunks, bias, KL ---
            for si, (F, O) in enumerate(STAGES, start=1):
                nw = F * O
                n_t = (nw + 127) // 128
                wmT = pp.tile([M, nw], FP, tag=f"wmT{si}", name=f"wmT{si}")
                mu_w = params[f"mu_w{si}"].ap().rearrange("f o m -> (f o) m")
                rho_w = params[f"rho_w{si}"].ap().rearrange("f o m -> (f o) m")
                eps_w = params[f"eps_w{si}"].ap().rearrange("f o m -> (f o) m")
                for t in range(n_t):
                    r0, r1 = t * 128, min((t + 1) * 128, nw)
                    rows = r1 - r0
                    tg = f"{si}_{t}"
                    mu_t = pp.tile([128, M], FP, tag=f"mu_t{tg}",
                                   name=f"mu_t{tg}")
                    rho_t = pp.tile([128, M], FP, tag=f"rho_t{tg}",
                                    name=f"rho_t{tg}")
                    eps_t = pp.tile([128, M], FP, tag=f"eps_t{tg}",
                                    name=f"eps_t{tg}")
                    nc.sync.dma_start(out=mu_t[0:rows, :], in_=mu_w[r0:r1, :])
                    nc.sync.dma_start(out=rho_t[0:rows, :],
                                      in_=rho_w[r0:r1, :])
                    nc.sync.dma_start(out=eps_t[0:rows, :],
                                      in_=eps_w[r0:r1, :])
                    sw_t = pp.tile([128, M], FP, tag=f"sw_t{tg}",
                                   name=f"sw_t{tg}")
                    # softplus(rho) = ln(1 + exp(rho))
                    nc.scalar.activation(sw_t[0:rows, :], rho_t[0:rows, :],
                                         AF.Exp)
                    nc.scalar.activation(sw_t[0:rows, :], sw_t[0:rows, :],
                                         AF.Ln, bias=1.0)
                    wm_t = pp.tile([128, M], FP, tag=f"wm_t{tg}",
                                   name=f"wm_t{tg}")
                    nc.vector.tensor_tensor(wm_t[0:rows, :], sw_t[0:rows, :],
                                            eps_t[0:rows, :], ALU.mult)
                    nc.vector.tensor_tensor(wm_t[0:rows, :], wm_t[0:rows, :],
                                            mu_t[0:rows, :], ALU.add)
                    _kl_accum(nc, pp, klv, mu_t, sw_t, rows, M)
                    wps = pps.tile([M, 128], FP, tag="ps", name=f"wps{tg}")
                    nc.tensor.transpose(wps[:, 0:rows], wm_t[0:rows, :],
                                        ident[0:rows, 0:rows])
                    nc.scalar.copy(wmT[:, r0:r1], wps[:, 0:rows])

                # all W' matmuls for this stage into one PSUM tile
                wpp = pps.tile([M, nw], FP, tag="ps", name=f"wpp{si}")
                for f in range(F):
                    nc.tensor.matmul(wpp[:, f * O:(f + 1) * O], rvT[:],
                                     wmT[:, f * O:(f + 1) * O],
                                     start=True, stop=True)
                stage_wp = []
                for c in range(F // 2):
                    wp_sb = wp_pool.tile([128, O], FR_K, tag=f"wp{si}_{c}",
                                         name=f"wp{si}_{c}")
                    nc.scalar.copy(wp_sb[:], zeros_t[:, 0:O])
                    for half in range(2):
                        f = 2 * c + half
                        nc.scalar.copy(wp_sb[half * 64:half * 64 + M, :],
                                       wpp[:, f * O:(f + 1) * O])
                    stage_wp.append(wp_sb)
                wp_tiles.append(stage_wp)

                # bias
                mu_b = pp.tile([F, O], FP, tag=f"mu_b{si}", name=f"mu_b{si}")
                rho_b = pp.tile([F, O], FP, tag=f"rho_b{si}",
                                name=f"rho_b{si}")
                eps_b = pp.tile([F, O], FP, tag=f"eps_b{si}",
                                name=f"eps_b{si}")
                nc.sync.dma_start(out=mu_b[:], in_=params[f"mu_b{si}"][:, :])
                nc.sync.dma_start(out=rho_b[:], in_=params[f"rho_b{si}"][:, :])
                nc.sync.dma_start(out=eps_b[:], in_=params[f"eps_b{si}"][:, :])
                sb_t = pp.tile([F, O], FP, tag=f"sb_t{si}", name=f"sb_t{si}")
                nc.scalar.activation(sb_t[:], rho_b[:], AF.Exp)
                nc.scalar.activation(sb_t[:], sb_t[:], AF.Ln, bias=1.0)
                bm = pp.tile([F, O], FP, tag=f"bm{si}", name=f"bm{si}")
                nc.vector.tensor_tensor(bm[:], sb_t[:], eps_b[:], ALU.mult)
                nc.vector.tensor_tensor(bm[:], bm[:], mu_b[:], ALU.add)
                _kl_accum(nc, pp, klv, mu_b, sb_t, F, O)
                bps = pps.tile([O, 1], FP, tag="ps", name=f"bps{si}")
                nc.tensor.matmul(bps[:], bm[:], ones_col[0:F, :],
                                 start=True, stop=True)
                bias_sb = const.tile([O, 1], FP, tag=f"bias{si}")
                nc.scalar.copy(bias_sb[:], bps[:])
                bias_tiles.append(bias_sb)

            # finalize KL
            klps = pps.tile([1, 1], FP, tag="ps", name="klps")
            nc.tensor.matmul(klps[:], klv[:], ones_col[:],
                             start=True, stop=True)
            kl_sb = pp.tile([1, 1], FP, tag="kl_sb")
            nc.scalar.activation(kl_sb[:], klps[:], AF.Identity,
                                 bias=kl_off[:])
            nc.sync.dma_start(out=kl[:, :], in_=kl_sb[:])

        # ---------- main stage loops ----------
        with ExitStack() as main_ctx:
            tp = main_ctx.enter_context(
                tc.tile_pool(name="tps", bufs=3, space="PSUM"))
            # (three [128,BB] slots rotate across the d/-d pairs)
            ops_pool = main_ctx.enter_context(
                tc.tile_pool(name="ops", bufs=1, space="PSUM"))
            absp = main_ctx.enter_context(tc.tile_pool(name="absp", bufs=3))
            negp = main_ctx.enter_context(tc.tile_pool(name="negp", bufs=3))
            kp = main_ctx.enter_context(tc.tile_pool(name="kp", bufs=4))

            h_in = h_tiles[0]
            gidx = 0  # global chunk counter (path split)
            for si, (F, O) in enumerate(STAGES):
                C = F // 2
                h_next = h3 if si == len(STAGES) - 1 else h_tiles[si + 1]
                for hb in range(BL // BB):
                    out_ps = ops_pool.tile([O, BB], FP, tag="out_ps")
                    for c in range(C):
                        ep = E_sb[0:33, c * 128:(c + 1) * 128]
                        d_t = tp.tile([128, BB], FP, tag="tps")
                        for j in range(BB // 512):
                            b0 = hb * BB + j * 512
                            nc.tensor.matmul(
                                d_t[:, j * 512:(j + 1) * 512],
                                ep, h_in[0:33, b0:b0 + 512],
                                start=True, stop=True)
                        k_t = kp.tile([128, BB], FR_K, tag="k_t")
                        if gidx % 3 < 2:
                            # DVE path: negd = -d; z = min(d, -d) = -|d|
                            negd = negp.tile([128, BB], FP, tag="negd")
                            nc.vector.tensor_scalar(
                                negd[:], d_t[:], -1.0, None, ALU.mult)
                            z_t = absp.tile([128, BB], FP, tag="abs_t")
                            nc.vector.tensor_tensor(z_t[:], d_t[:],
                                                    negd[:], ALU.min)
                            nc.scalar.activation(k_t[:], z_t[:], AF.Exp)
                        else:
                            # ACT path: a = |d|; k = exp(-a)
                            a_t = absp.tile([128, BB], FP, tag="abs_t")
                            nc.scalar.activation(a_t[:], d_t[:], AF.Abs)
                            nc.scalar.activation(k_t[:], a_t[:], AF.Exp,
                                                 scale=-1.0)
                        gidx += 1
                        for j in range(BB // 512):
                            nc.tensor.matmul(
                                out_ps[:, j * 512:(j + 1) * 512],
                                wp_tiles[si][c][:],
                                k_t[:, j * 512:(j + 1) * 512],
                                start=(c == 0), stop=(c == C - 1))
                    hv = (h_next[:, hb * BB:(hb + 1) * BB]
                          if si == len(STAGES) - 1 else
                          h_next[0:O, hb * BB:(hb + 1) * BB])
                    nc.scalar.activation(hv, out_ps[:], AF.Identity,
                                         bias=bias_tiles[si][:])
                h_in = h_next

            nc.sync.dma_start(out=out[:, :], in_=h3[:])


def _kl_accum(nc, pp, klv, mu_t, sig_t, rows, width):
    """klv[0:rows] += sum_j 0.5*(sig^2 + mu^2) - ln(sig) over free dim."""
    n = _KL_N[0] = _KL_N[0] + 1
    scratch = pp.tile([128, width], FP, tag=f"kl_scratch{n}",
                      name=f"kl_scratch{n}")
    acc_s = pp.tile([128, 1], FP, tag=f"kl_acc_s{n}", name=f"kl_acc_s{n}")
    acc_m = pp.tile([128, 1], FP, tag=f"kl_acc_m{n}", name=f"kl_acc_m{n}")
    acc_l = pp.tile([128, 1], FP, tag=f"kl_acc_l{n}", name=f"kl_acc_l{n}")
    nc.scalar.activation(scratch[0:rows, :], sig_t[0:rows, :], AF.Square,
                         accum_out=acc_s[0:rows, :])
    nc.scalar.activation(scratch[0:rows, :], mu_t[0:rows, :], AF.Square,
                         accum_out=acc_m[0:rows, :])
    nc.scalar.activation(scratch[0:rows, :], sig_t[0:rows, :], AF.Ln,
                         accum_out=acc_l[0:rows, :])
    tmp = pp.tile([128, 1], FP, tag=f"kl_tmp{n}", name=f"kl_tmp{n}")
    nc.vector.tensor_tensor(tmp[0:rows, :], acc_s[0:rows, :],
                            acc_m[0:rows, :], ALU.add)
    nc.vector.tensor_scalar(tmp[0:rows, :], tmp[0:rows, :], 0.5, None,
                            ALU.mult)
    nc.vector.tensor_tensor(tmp[0:rows, :], tmp[0:rows, :], acc_l[0:rows, :],
                            ALU.subtract)
    nc.vector.tensor_tensor(klv[0:rows, :], klv[0:rows, :], tmp[0:rows, :],
                            ALU.add)


_KL_N = [0]
_NC_CACHE = {}


def get_nc(debug=False):
    if debug not in _NC_CACHE:
        _KL_N[0] = 0
        _NC_CACHE[debug] = build_nc(debug=debug)
    return _NC_CACHE[debug]


def make_in_maps(inputs):
    x_full = np.ascontiguousarray(inputs["x"], dtype=np.float32)
    shared = {k: np.ascontiguousarray(v, dtype=np.float32)
              for k, v in inputs.items() if k not in ("x", "U")}
    shared.update(host_constants())
    shared["E_all"] = _e_all(np.asarray(inputs["U"], dtype=np.float32))
    return [dict(shared, x=x_full[i * BL:(i + 1) * BL])
            for i in range(N_CORES)]


def kernel(**inputs):
    """Takes FULL unsharded inputs, returns (out [16384], kl scalar)."""
    nc = get_nc()
    in_maps = make_in_maps(inputs)
    res = bass_utils.run_bass_kernel_spmd(nc, in_maps,
                                          core_ids=list(range(N_CORES)))
    outs = np.concatenate(
        [res.results[i]["out"].reshape(-1) for i in range(N_CORES)])
    kl_val = np.float32(res.results[0]["kl"].reshape(-1)[0])
    return outs, kl_val


# revision 48
# speedup vs baseline: 1.3467x; 1.3467x over previous
"""Trainium2 Bass kernel for nn_AdditiveDTMGP.

3-stage additive GP network. Per stage:
    k = exp(-|h[b,f] - U[m]|)           ([B,F,63])
    feat = k @ Rinv
    out[b,o] = einsum('bfn,fon->bo', feat, Wm) + bias,
    Wm = mu + softplus(rho)*eps
plus a scalar KL over the variational params.

Key algebraic rewrite: fold Rinv into the weights
    out[b,o] = sum_{f,m} k[b,f,m] * W'[f,o,m]
    W'[f,o,m] = sum_n Rinv[m,n] * Wm[f,o,n]
which eliminates the [B,F,63]x[63,63] feature einsum entirely.

On-chip layout per stage (per core, B_local = 2048):
  - chunks of 128 partitions: feature 2c on p in [0,63), feature 2c+1 on
    p in [64,127), rows 63/127 dead (W' rows zeroed)
  - PE broadcast-matmul (host-built selector E) replicates h^T rows -> PSUM
  - DVE fused tensor_scalar: (t - U[p]) then abs_max(.,0) = |t - U|
  - ACT Exp(scale=-1): k = exp(-|t-U|)
  - PE contraction matmul (k x W'_chunk) accumulated over chunks in PSUM
  - ACT Identity(bias) evacuates PSUM -> h_next (bias folded in)

Sharding: pure data parallelism over batch (16384 -> 8 x 2048), params
replicated; KL computed redundantly per core (core 0's value returned).
"""

from contextlib import ExitStack

import ml_dtypes
import numpy as np

import concourse.bacc as bacc
import concourse.bass as bass
import concourse.mybir as mybir
import concourse.tile as tile
from concourse import bass_utils

# ---- problem constants (hardcoded; kernel.py must be self-contained) ----
B_FULL = 16384
N_CORES = 8
BL = B_FULL // N_CORES  # 2048 batch per core
M = 63                  # dyadic design points
BB = 1024               # batch block size in the main loop
STAGES = [(32, 16), (16, 16), (16, 1)]  # (F, O)
FP = mybir.dt.float32
BF = mybir.dt.bfloat16  # matmul operands as bf16 hi/lo pairs (near-exact:
                        # E and U entries are exactly representable; h and
                        # W' are split into hi+lo; k is single-bf16)

AF = mybir.ActivationFunctionType
ALU = mybir.AluOpType

N_KL = sum(F * O * M + F * O for F, O in STAGES)


def host_constants():
    """Constant input tensors computed on the host (replicated per core)."""
    ident = np.eye(128, dtype=np.float32)
    ones_col = np.ones((128, 1), dtype=np.float32)
    zeros_t = np.zeros((128, 16), dtype=np.float32)
    kl_off = np.full((1, 1), -0.5 * N_KL, dtype=np.float32)
    return {"ident": ident, "ones_col": ones_col,
            "zeros_t": zeros_t, "kl_off": kl_off}


def _e_all(U):
    """Paired broadcast selectors: cols [0:2048] produce d = h_f - U[m],
    cols [2048:4096] produce -d. Row 32 multiplies the constant ones row
    of h; rows 16..31 are zero for the F=16 stages (h junk killers)."""
    u_col = np.zeros(128, dtype=np.float32)
    u_col[0:M] = U
    u_col[64:64 + M] = U
    e = np.zeros((33, 16 * 128), dtype=np.float32)
    for c in range(16):
        e[2 * c, c * 128:c * 128 + M] = 1.0
        e[2 * c + 1, c * 128 + 64:c * 128 + 64 + M] = 1.0
        e[32, c * 128:(c + 1) * 128] = -u_col
    return e


def build_nc(debug: bool = False) -> bass.Bass:
    nc = bacc.Bacc("TRN2", debug=debug, target_bir_lowering=False,
                   num_devices=N_CORES)

    x = nc.dram_tensor("x", [BL, 32], FP, kind="ExternalInput")
    Rinv = nc.dram_tensor("Rinv", [M, M], FP, kind="ExternalInput")
    consts = {
        "E_all": nc.dram_tensor("E_all", [33, 16 * 128], BF,
                                kind="ExternalInput"),
        "ident": nc.dram_tensor("ident", [128, 128], FP,
                                kind="ExternalInput"),
        "ones_col": nc.dram_tensor("ones_col", [128, 1], FP,
                                   kind="ExternalInput"),
        "zeros_t": nc.dram_tensor("zeros_t", [128, 16], FP,
                                  kind="ExternalInput"),
        "kl_off": nc.dram_tensor("kl_off", [1, 1], FP,
                                 kind="ExternalInput"),
    }
    params = {}
    for si, (F, O) in enumerate(STAGES, start=1):
        for nm in ("mu_w", "rho_w", "eps_w"):
            params[f"{nm}{si}"] = nc.dram_tensor(
                f"{nm}{si}", [F, O, M], FP, kind="ExternalInput")
        for nm in ("mu_b", "rho_b", "eps_b"):
            params[f"{nm}{si}"] = nc.dram_tensor(
                f"{nm}{si}", [F, O], FP, kind="ExternalInput")
    out = nc.dram_tensor("out", [1, BL], FP, kind="ExternalOutput")
    kl = nc.dram_tensor("kl", [1, 1], FP, kind="ExternalOutput")

    with tile.TileContext(nc) as tc:
        _build(tc, x, Rinv, consts, params, out, kl)
    nc.compile()
    return nc


def _build(tc, x, Rinv, consts, params, out, kl):
    nc = tc.nc
    with ExitStack() as ctx:
        const = ctx.enter_context(tc.tile_pool(name="const", bufs=1))
        hbuf = ctx.enter_context(tc.tile_pool(name="hbuf", bufs=1))
        wp_pool = ctx.enter_context(tc.tile_pool(name="wp", bufs=1))
        pp = ctx.enter_context(tc.tile_pool(name="prep", bufs=1))

        # ---------- host constants -> SBUF ----------
        E_sb = const.tile([33, 16 * 128], BF, tag="E_sb")
        nc.sync.dma_start(out=E_sb[:], in_=consts["E_all"][:, :])
        ident = const.tile([128, 128], FP, tag="ident")
        nc.sync.dma_start(out=ident[:], in_=consts["ident"][:, :])
        ones_col = const.tile([128, 1], FP, tag="ones_col")
        nc.sync.dma_start(out=ones_col[:], in_=consts["ones_col"][:, :])
        zeros_t = const.tile([128, 16], FP, tag="zeros_t")
        nc.sync.dma_start(out=zeros_t[:], in_=consts["zeros_t"][:, :])
        kl_off = const.tile([1, 1], FP, tag="kl_off")
        nc.sync.dma_start(out=kl_off[:], in_=consts["kl_off"][:, :])

        # klv accumulates per-partition KL partial sums
        klv = const.tile([128, 1], FP, tag="klv")
        nc.vector.tensor_copy(klv[:], zeros_t[:, 0:1])

        # persistent stage activations (h0 = x^T): fp32 canonical plus
        # bf16 hi/lo pairs (h = hi + lo to 2^-17) fed to the broadcasts.
        # Row 32 is a constant ones row (multiplied by E's -U row); rows
        # F..31 are zeroed so the E selectors see no garbage.
        h_tiles = [hbuf.tile([33, BL], FP, tag=f"h{si}", name=f"h{si}")
                   for si in range(len(STAGES))]
        h_hi = [hbuf.tile([33, BL], BF, tag=f"hh{si}", name=f"hh{si}")
                for si in range(len(STAGES))]
        h_lo = [hbuf.tile([33, BL], BF, tag=f"hl{si}", name=f"hl{si}")
                for si in range(len(STAGES))]
        h3 = hbuf.tile([1, BL], FP, tag="h3")
        for ht in h_tiles[1:] + h_hi[1:] + h_lo[1:]:
            nc.vector.memset(ht[:], 0.0)
        for ht in h_lo[:1]:
            nc.vector.memset(ht[:], 0.0)
        for ht in h_tiles:
            nc.vector.memset(ht[32:33, :], 1.0)
        for ht in h_hi:
            nc.vector.memset(ht[32:33, :], 1.0)

        wp_tiles = []    # per stage: list of [128, O] chunk weight tiles
        bias_tiles = []  # per stage: [O, 1]

        with ExitStack() as prep_ctx:
            pps = prep_ctx.enter_context(
                tc.tile_pool(name="prep_ps", bufs=3, space="PSUM"))

            # --- x^T via PE transposes (one DMA, [p, (i f)] layout) ---
            xT = h_tiles[0]
            x_big = pp.tile([128, 16 * 32], FP, tag="x_big")
            # x_big[p, i*32+f] = x[i*128+p, f]
            nc.sync.dma_start(
                out=x_big[:],
                in_=bass.AP(x, 0, [[32, 128], [128 * 32, 16], [1, 32]]))
            for g in range(4):  # 4 psum tiles, 4 transposes each
                xps = pps.tile([32, 512], FP, tag="ps", name=f"xps{g}")
                for t in range(4):
                    i = g * 4 + t
                    nc.tensor.transpose(xps[:, t * 128:(t + 1) * 128],
                                        x_big[:, i * 32:(i + 1) * 32],
                                        ident[:])
                sl = slice(g * 512, (g + 1) * 512)
                nc.scalar.copy(xT[0:32, sl], xps[:])
                nc.scalar.copy(h_hi[0][0:32, sl], xps[:])
                nc.vector.tensor_tensor(h_lo[0][0:32, sl], xps[:],
                                        h_hi[0][0:32, sl], ALU.subtract)

            # --- Rinv^T ---
            rv = pp.tile([M, M], FP, tag="rv")
            nc.sync.dma_start(out=rv[:], in_=Rinv[:, :])
            rps = pps.tile([M, M], FP, tag="ps", name="rps")
            nc.tensor.transpose(rps[:], rv[:], ident[0:M, 0:M])
            rvT = const.tile([M, M], FP, tag="rvT")
            nc.scalar.copy(rvT[:], rps[:])

            # --- per stage: W' chunks, bias, KL ---
            for si, (F, O) in enumerate(STAGES, start=1):
                nw = F * O
                n_t = (nw + 127) // 128
                wmT = pp.tile([M, nw], FP, tag=f"wmT{si}", name=f"wmT{si}")
                mu_w = params[f"mu_w{si}"].ap().rearrange("f o m -> (f o) m")
                rho_w = params[f"rho_w{si}"].ap().rearrange("f o m -> (f o) m")
                eps_w = params[f"eps_w{si}"].ap().rearrange("f o m -> (f o) m")
                for t in range(n_t):
                    r0, r1 = t * 128, min((t + 1) * 128, nw)
                    rows = r1 - r0
                    tg = f"{si}_{t}"
                    mu_t = pp.tile([128, M], FP, tag=f"mu_t{tg}",
                                   name=f"mu_t{tg}")
                    rho_t = pp.tile([128, M], FP, tag=f"rho_t{tg}",
                                    name=f"rho_t{tg}")
                    eps_t = pp.tile([128, M], FP, tag=f"eps_t{tg}",
                                    name=f"eps_t{tg}")
                    nc.sync.dma_start(out=mu_t[0:rows, :], in_=mu_w[r0:r1, :])
                    nc.sync.dma_start(out=rho_t[0:rows, :],
                                      in_=rho_w[r0:r1, :])
                    nc.sync.dma_start(out=eps_t[0:rows, :],
                                      in_=eps_w[r0:r1, :])
                    sw_t = pp.tile([128, M], FP, tag=f"sw_t{tg}",
                                   name=f"sw_t{tg}")
                    # softplus(rho) = ln(1 + exp(rho))
                    nc.scalar.activation(sw_t[0:rows, :], rho_t[0:rows, :],
                                         AF.Exp)
                    nc.scalar.activation(sw_t[0:rows, :], sw_t[0:rows, :],
                                         AF.Ln, bias=1.0)
                    wm_t = pp.tile([128, M], FP, tag=f"wm_t{tg}",
                                   name=f"wm_t{tg}")
                    nc.vector.tensor_tensor(wm_t[0:rows, :], sw_t[0:rows, :],
                                            eps_t[0:rows, :], ALU.mult)
                    nc.vector.tensor_tensor(wm_t[0:rows, :], wm_t[0:rows, :],
                                            mu_t[0:rows, :], ALU.add)
                    _kl_accum(nc, pp, klv, mu_t, sw_t, rows, M)
                    wps = pps.tile([M, 128], FP, tag="ps", name=f"wps{tg}")
                    nc.tensor.transpose(wps[:, 0:rows], wm_t[0:rows, :],
                                        ident[0:rows, 0:rows])
                    nc.scalar.copy(wmT[:, r0:r1], wps[:, 0:rows])

                # all W' matmuls for this stage into one PSUM tile
                wpp = pps.tile([M, nw], FP, tag="ps", name=f"wpp{si}")
                for f in range(F):
                    nc.tensor.matmul(wpp[:, f * O:(f + 1) * O], rvT[:],
                                     wmT[:, f * O:(f + 1) * O],
                                     start=True, stop=True)
                stage_wp = []
                for c in range(F // 2):
                    wp_sb = wp_pool.tile([128, O], FP, tag=f"wp{si}_{c}",
                                         name=f"wp{si}_{c}")
                    nc.scalar.copy(wp_sb[:], zeros_t[:, 0:O])
                    for half in range(2):
                        f = 2 * c + half
                        nc.scalar.copy(wp_sb[half * 64:half * 64 + M, :],
                                       wpp[:, f * O:(f + 1) * O])
                    stage_wp.append(wp_sb)
                wp_tiles.append(stage_wp)

                # bias
                mu_b = pp.tile([F, O], FP, tag=f"mu_b{si}", name=f"mu_b{si}")
                rho_b = pp.tile([F, O], FP, tag=f"rho_b{si}",
                                name=f"rho_b{si}")
                eps_b = pp.tile([F, O], FP, tag=f"eps_b{si}",
                                name=f"eps_b{si}")
                nc.sync.dma_start(out=mu_b[:], in_=params[f"mu_b{si}"][:, :])
                nc.sync.dma_start(out=rho_b[:], in_=params[f"rho_b{si}"][:, :])
                nc.sync.dma_start(out=eps_b[:], in_=params[f"eps_b{si}"][:, :])
                sb_t = pp.tile([F, O], FP, tag=f"sb_t{si}", name=f"sb_t{si}")
                nc.scalar.activation(sb_t[:], rho_b[:], AF.Exp)
                nc.scalar.activation(sb_t[:], sb_t[:], AF.Ln, bias=1.0)
                bm = pp.tile([F, O], FP, tag=f"bm{si}", name=f"bm{si}")
                nc.vector.tensor_tensor(bm[:], sb_t[:], eps_b[:], ALU.mult)
                nc.vector.tensor_tensor(bm[:], bm[:], mu_b[:], ALU.add)
                _kl_accum(nc, pp, klv, mu_b, sb_t, F, O)
                bps = pps.tile([O, 1], FP, tag="ps", name=f"bps{si}")
                nc.tensor.matmul(bps[:], bm[:], ones_col[0:F, :],
                                 start=True, stop=True)
                bias_sb = const.tile([O, 1], FP, tag=f"bias{si}")
                nc.scalar.copy(bias_sb[:], bps[:])
                bias_tiles.append(bias_sb)

            # finalize KL
            klps = pps.tile([1, 1], FP, tag="ps", name="klps")
            nc.tensor.matmul(klps[:], klv[:], ones_col[:],
                             start=True, stop=True)
            kl_sb = pp.tile([1, 1], FP, tag="kl_sb")
            nc.scalar.activation(kl_sb[:], klps[:], AF.Identity,
                                 bias=kl_off[:])
            nc.sync.dma_start(out=kl[:, :], in_=kl_sb[:])

        # ---------- main stage loops ----------
        with ExitStack() as main_ctx:
            tp = main_ctx.enter_context(
                tc.tile_pool(name="tps", bufs=3, space="PSUM"))
            # (three [128,BB] slots rotate across the d/-d pairs)
            ops_pool = main_ctx.enter_context(
                tc.tile_pool(name="ops", bufs=1, space="PSUM"))
            absp = main_ctx.enter_context(tc.tile_pool(name="absp", bufs=3))
            negp = main_ctx.enter_context(tc.tile_pool(name="negp", bufs=3))
            kp = main_ctx.enter_context(tc.tile_pool(name="kp", bufs=4))

            gidx = 0  # global chunk counter (path split)
            for si, (F, O) in enumerate(STAGES):
                C = F // 2
                hi_in, lo_in = h_hi[si], h_lo[si]
                h_next = h3 if si == len(STAGES) - 1 else h_tiles[si + 1]
                for hb in range(BL // BB):
                    out_ps = ops_pool.tile([O, BB], FP, tag="out_ps")
                    for c in range(C):
                        ep = E_sb[0:33, c * 128:(c + 1) * 128]
                        d_t = tp.tile([128, BB], FP, tag="tps")
                        for j in range(BB // 512):
                            b0 = hb * BB + j * 512
                            nc.tensor.matmul(
                                d_t[:, j * 512:(j + 1) * 512],
                                ep, hi_in[0:33, b0:b0 + 512],
                                start=True, stop=False)
                            nc.tensor.matmul(
                                d_t[:, j * 512:(j + 1) * 512],
                                ep, lo_in[0:33, b0:b0 + 512],
                                start=False, stop=True)
                        k_t = kp.tile([128, BB], FP, tag="k_t")
                        if gidx % 3 < 2:
                            # DVE path: negd = -d; z = min(d, -d) = -|d|
                            negd = negp.tile([128, BB], FP, tag="negd")
                            nc.vector.tensor_scalar(
                                negd[:], d_t[:], -1.0, None, ALU.mult)
                            z_t = absp.tile([128, BB], FP, tag="abs_t")
                            nc.vector.tensor_tensor(z_t[:], d_t[:],
                                                    negd[:], ALU.min)
                            nc.scalar.activation(k_t[:], z_t[:], AF.Exp)
                        else:
                            # ACT path: a = |d|; k = exp(-a)
                            a_t = absp.tile([128, BB], FP, tag="abs_t")
                            nc.scalar.activation(a_t[:], d_t[:], AF.Abs)
                            nc.scalar.activation(k_t[:], a_t[:], AF.Exp,
                                                 scale=-1.0)
                        gidx += 1
                        for j in range(BB // 512):
                            nc.tensor.matmul(
                                out_ps[:, j * 512:(j + 1) * 512],
                                wp_tiles[si][c][:],
                                k_t[:, j * 512:(j + 1) * 512],
                                start=(c == 0), stop=(c == C - 1))
                    blk = slice(hb * BB, (hb + 1) * BB)
                    hv = (h_next[:, blk] if si == len(STAGES) - 1
                          else h_next[0:O, blk])
                    nc.scalar.activation(hv, out_ps[:], AF.Identity,
                                         bias=bias_tiles[si][:])
                    if si < len(STAGES) - 1:
                        nhi, nlo = h_hi[si + 1], h_lo[si + 1]
                        nc.scalar.copy(nhi[0:O, blk], h_next[0:O, blk])
                        nc.vector.tensor_tensor(nlo[0:O, blk],
                                                h_next[0:O, blk],
                                                nhi[0:O, blk],
                                                ALU.subtract)

            nc.sync.dma_start(out=out[:, :], in_=h3[:])


def _kl_accum(nc, pp, klv, mu_t, sig_t, rows, width):
    """klv[0:rows] += sum_j 0.5*(sig^2 + mu^2) - ln(sig) over free dim."""
    n = _KL_N[0] = _KL_N[0] + 1
    scratch = pp.tile([128, width], FP, tag=f"kl_scratch{n}",
                      name=f"kl_scratch{n}")
    acc_s = pp.tile([128, 1], FP, tag=f"kl_acc_s{n}", name=f"kl_acc_s{n}")
    acc_m = pp.tile([128, 1], FP, tag=f"kl_acc_m{n}", name=f"kl_acc_m{n}")
    acc_l = pp.tile([128, 1], FP, tag=f"kl_acc_l{n}", name=f"kl_acc_l{n}")
    nc.scalar.activation(scratch[0:rows, :], sig_t[0:rows, :], AF.Square,
                         accum_out=acc_s[0:rows, :])
    nc.scalar.activation(scratch[0:rows, :], mu_t[0:rows, :], AF.Square,
                         accum_out=acc_m[0:rows, :])
    nc.scalar.activation(scratch[0:rows, :], sig_t[0:rows, :], AF.Ln,
                         accum_out=acc_l[0:rows, :])
    tmp = pp.tile([128, 1], FP, tag=f"kl_tmp{n}", name=f"kl_tmp{n}")
    nc.vector.tensor_tensor(tmp[0:rows, :], acc_s[0:rows, :],
                            acc_m[0:rows, :], ALU.add)
    nc.vector.tensor_scalar(tmp[0:rows, :], tmp[0:rows, :], 0.5, None,
                            ALU.mult)
    nc.vector.tensor_tensor(tmp[0:rows, :], tmp[0:rows, :], acc_l[0:rows, :],
                            ALU.subtract)
    nc.vector.tensor_tensor(klv[0:rows, :], klv[0:rows, :], tmp[0:rows, :],
                            ALU.add)


_KL_N = [0]
_NC_CACHE = {}


def get_nc(debug=False):
    if debug not in _NC_CACHE:
        _KL_N[0] = 0
        _NC_CACHE[debug] = build_nc(debug=debug)
    return _NC_CACHE[debug]


def make_in_maps(inputs):
    x_full = np.ascontiguousarray(inputs["x"], dtype=np.float32)
    shared = {k: np.ascontiguousarray(v, dtype=np.float32)
              for k, v in inputs.items() if k not in ("x", "U")}
    shared.update(host_constants())
    shared["E_all"] = _e_all(np.asarray(inputs["U"], dtype=np.float32)
                             ).astype(ml_dtypes.bfloat16)
    return [dict(shared, x=x_full[i * BL:(i + 1) * BL])
            for i in range(N_CORES)]


def kernel(**inputs):
    """Takes FULL unsharded inputs, returns (out [16384], kl scalar)."""
    nc = get_nc()
    in_maps = make_in_maps(inputs)
    res = bass_utils.run_bass_kernel_spmd(nc, in_maps,
                                          core_ids=list(range(N_CORES)))
    outs = np.concatenate(
        [res.results[i]["out"].reshape(-1) for i in range(N_CORES)])
    kl_val = np.float32(res.results[0]["kl"].reshape(-1)[0])
    return outs, kl_val
